# revision 3
# baseline (speedup 1.0000x reference)
"""GCN layer kernel for Trainium2, 8-core SPMD.

Computes: out = (A @ (X @ W + b)) / colsum(A)[:, None],  A = (adj != 0)
with N=8192 nodes, F_in=F_out=512, across 8 NeuronCores.

Sharding: row-shard adjacency and node features (1024 rows per core),
replicate W/b. Each core computes its projected-hidden block, all-gathers
the full hidden, then owns its output row block. Degree (column sums of A)
needs rows from every core, so each core emits partial column sums
(computed for free via the binarize op's accumulate output) and a second
all-gather + on-device summation produces the full degree vector.

All matmuls run in float32r (FP22-truncated fp32): full PE throughput at
N=512 moving dim with ~13-bit mantissa accuracy.
"""
import numpy as np

N = 8192
F = 512
N_CORES = 8
NB = N // N_CORES          # 1024 rows per core
KT = N // 128              # 64 contraction tiles
MT = NB // 128             # 8 output row tiles per core
FI_T = F // 128            # 4 feat-in tiles

_cached = {}


def _build():
    import concourse.bacc as bacc
    import concourse.bass as bass
    import concourse.tile as tile
    from concourse import mybir

    f32 = mybir.dt.float32
    f32r = mybir.dt.float32r

    nc = bacc.Bacc("TRN2", target_bir_lowering=False, debug=False,
                   num_devices=N_CORES)
    at = nc.dram_tensor("at", [N, NB], f32, kind="ExternalInput").ap()
    xt = nc.dram_tensor("xt", [F, NB], f32, kind="ExternalInput").ap()
    w = nc.dram_tensor("w", [F, F], f32, kind="ExternalInput").ap()
    bfull = nc.dram_tensor("bfull", [128, F], f32, kind="ExternalInput").ap()
    out = nc.dram_tensor("out", [NB, F], f32, kind="ExternalOutput").ap()

    pid = nc.partition_id()

    with tile.TileContext(nc) as tc:
        with tc.tile_pool(name="dram", bufs=1, space="DRAM") as dram, \
             tc.tile_pool(name="persist", bufs=1) as pp:
            hg_in = dram.tile([NB, F], f32r)
            hg_out = dram.tile([N, F], f32r, addr_space="Shared")
            dg_in = dram.tile([128, KT], f32)
            dg_out = dram.tile([128 * N_CORES, KT], f32, addr_space="Shared")

            cs = pp.tile([128, KT], f32)   # per-core partial column sums

            # ---- phase 1: H_blk = X_blk @ W + b  (f32r matmuls) ----
            with tc.tile_pool(name="ph1", bufs=1) as p1, \
                 tc.tile_pool(name="ph1ps", bufs=2, space="PSUM") as ps1:
                xt_sb = p1.tile([128, FI_T * NB], f32)
                w_sb = p1.tile([128, FI_T * F], f32)
                for ki in range(FI_T):
                    nc.sync.dma_start(xt_sb[:, ki * NB:(ki + 1) * NB],
                                      at_rows(xt, ki))
                    nc.sync.dma_start(w_sb[:, ki * F:(ki + 1) * F],
                                      at_rows(w, ki))
                b_sb = p1.tile([128, F], f32)
                nc.sync.dma_start(b_sb[:], bfull)
                # round fp32 -> fp32r for the PE
                xt_r = p1.tile([128, FI_T * NB], f32r)
                nc.vector.tensor_copy(xt_r[:], xt_sb[:])
                w_r = p1.tile([128, FI_T * F], f32r)
                nc.vector.tensor_copy(w_r[:], w_sb[:])

                for nt in range(MT):
                    hp = ps1.tile([128, F], f32, tag="hp", bufs=2)
                    for ki in range(FI_T):
                        nc.tensor.matmul(
                            hp[:],
                            xt_r[:, ki * NB + nt * 128: ki * NB + (nt + 1) * 128],
                            w_r[:, ki * F:(ki + 1) * F],
                            start=(ki == 0), stop=(ki == FI_T - 1))
                    hb = p1.tile([128, F], f32r, tag="hb", bufs=2)
                    nc.vector.tensor_tensor(hb[:], hp[:], b_sb[:],
                                            mybir.AluOpType.add)
                    nc.sync.dma_start(hg_in[nt * 128:(nt + 1) * 128, :], hb[:])

            # ---- all-gather projected hidden ----
            nc.gpsimd.collective_compute(
                "AllGather", mybir.AluOpType.bypass,
                replica_groups=[list(range(N_CORES))],
                ins=[hg_in.opt()], outs=[hg_out.opt()],
            )

            # ---- phase 2: out_psum[m] += A_bin_kt[:, m].T @ H_kt ----
            with tc.tile_pool(name="ph2", bufs=1) as p2, \
                 tc.tile_pool(name="ph2ps", bufs=1, space="PSUM") as ps2:
                ones = p2.tile([128, NB], f32)
                nc.vector.memset(ones[:], 1.0)
                pms = []
                for m in range(MT):
                    pm = ps2.tile([128, F], f32, tag=f"pm{m}", name=f"pm{m}",
                                  bufs=1)
                    pms.append(pm)
                for kt in range(KT):
                    a_raw = p2.tile([128, NB], f32, tag="araw", bufs=6)
                    nc.sync.dma_start(a_raw[:],
                                      at[kt * 128:(kt + 1) * 128, :])
                    # one DVE op: a_bin = (a_raw != 0) * 1.0 (rounded f32r),
                    # accum_out = free-dim sums = partial column sums of A
                    a_bin = p2.tile([128, NB], f32r, tag="abin", bufs=12)
                    nc.vector.scalar_tensor_tensor(
                        a_bin[:], a_raw[:], 0.0, ones[:],
                        mybir.AluOpType.not_equal, mybir.AluOpType.mult,
                        accum_out=cs[:, kt:kt + 1])
                    h_t = p2.tile([128, F], f32r, tag="ht", bufs=6)
                    nc.sync.dma_start(h_t[:],
                                      hg_out[kt * 128:(kt + 1) * 128, :])
                    for m in range(MT):
                        nc.tensor.matmul(
                            pms[m][:],
                            a_bin[:, m * 128:(m + 1) * 128],
                            h_t[:],
                            start=(kt == 0), stop=(kt == KT - 1))

                # ---- phase 3: degree + normalize ----
                nc.sync.dma_start(dg_in[:], cs[:])
                nc.gpsimd.collective_compute(
                    "AllGather", mybir.AluOpType.bypass,
                    replica_groups=[list(range(N_CORES))],
                    ins=[dg_in.opt()], outs=[dg_out.opt()],
                )
                # pull each rank's partial for OUR column block (kt = pid*8+m)
                deg = p2.tile([128, MT], f32)
                prt0 = p2.tile([128, MT], f32, tag="prt", bufs=4, name="prt0")
                nc.sync.dma_start(
                    prt0[:], dg_out[0:128, bass.ts(pid, MT)])
                nc.vector.tensor_copy(deg[:], prt0[:])
                for r in range(1, N_CORES):
                    prt = p2.tile([128, MT], f32, tag="prt", bufs=4,
                                  name=f"prt{r}")
                    nc.sync.dma_start(
                        prt[:],
                        dg_out[r * 128:(r + 1) * 128, bass.ts(pid, MT)])
                    nc.vector.tensor_tensor(deg[:], deg[:], prt[:],
                                            mybir.AluOpType.add)
                rdeg = p2.tile([128, MT], f32)
                nc.vector.reciprocal(rdeg[:], deg[:])

                for m in range(MT):
                    o_sb = p2.tile([128, F], f32, tag="osb", bufs=2,
                                   name=f"osb{m}")
                    nc.scalar.mul(o_sb[:], pms[m][:], rdeg[:, m:m + 1])
                    nc.sync.dma_start(out[m * 128:(m + 1) * 128, :], o_sb[:])

    nc.compile()
    return nc


def at_rows(ap, ki):
    """rows [ki*128, (ki+1)*128) of a DRAM AP."""
    return ap[ki * 128:(ki + 1) * 128, :]


def _get_nc():
    if "nc" not in _cached:
        _cached["nc"] = _build()
    return _cached["nc"]


def kernel(input_features, adj, W, b):
    from concourse.bass_utils import run_bass_kernel_spmd

    x = np.ascontiguousarray(np.asarray(input_features, dtype=np.float32))
    a = np.asarray(adj, dtype=np.float32)
    wm = np.ascontiguousarray(np.asarray(W, dtype=np.float32))
    bv = np.asarray(b, dtype=np.float32)
    bfull = np.ascontiguousarray(np.broadcast_to(bv, (128, F)))

    nc = _get_nc()
    in_maps = []
    for k in range(N_CORES):
        blk = slice(k * NB, (k + 1) * NB)
        in_maps.append({
            "at": np.ascontiguousarray(a[blk, :].T),
            "xt": np.ascontiguousarray(x[blk, :].T),
            "w": wm,
            "bfull": bfull,
        })
    res = run_bass_kernel_spmd(nc, in_maps, core_ids=list(range(N_CORES)))
    return np.concatenate([res.results[k]["out"] for k in range(N_CORES)],
                          axis=0)


# revision 7
# speedup vs baseline: 1.2567x; 1.2567x over previous
"""GCN layer kernel for Trainium2, 8-core SPMD.

Computes: out = (A @ (X @ W + b)) / colsum(A)[:, None],  A = (adj != 0)
with N=8192 nodes, F_in=F_out=512, across 8 NeuronCores.

Sharding: row-shard adjacency and node features (1024 rows per core),
replicate W/b. Each core computes its projected-hidden block in f32r
(FP22) precision, all-gathers the full hidden (bf16) across the chip,
then owns its output row block. Degree (column sums of A) needs rows
from every core: each core computes partial column sums for free via the
binarize op's accumulate output, a second all-gather shares them, and a
small on-device tree sum + reciprocal finishes the normalization.

PE warmth: the AllGather stalls the main matmuls ~60-100us after a short
phase 1; without countermeasures the HAM clock gate drops the PE to 1.2
GHz and the first ~half of the main loop runs at half throughput. We
issue cheap dummy matmuls paced by the A-tile DMA stream (a real data
dependency, so they spread out in time) to hold the PE at 2.4 GHz
through the gather window.
"""
import numpy as np

N = 8192
F = 512
N_CORES = 8
NB = N // N_CORES          # 1024 rows per core
KT = N // 128              # 64 contraction tiles
MT = NB // 128             # 8 output row tiles per core
FI_T = F // 128            # 4 feat-in tiles
N_DUMMY = 90               # junk warm-up matmuls (~0.9us each, f32 4-pass)

_cached = {}


def _build():
    import concourse.bacc as bacc
    import concourse.bass as bass
    import concourse.tile as tile
    from concourse import mybir

    f32 = mybir.dt.float32
    f32r = mybir.dt.float32r
    bf16 = mybir.dt.bfloat16

    nc = bacc.Bacc("TRN2", target_bir_lowering=False, debug=False,
                   num_devices=N_CORES)
    at = nc.dram_tensor("at", [N, NB], f32, kind="ExternalInput").ap()
    xt = nc.dram_tensor("xt", [F, NB], f32, kind="ExternalInput").ap()
    w = nc.dram_tensor("w", [F, F], f32, kind="ExternalInput").ap()
    bfull = nc.dram_tensor("bfull", [128, F], f32, kind="ExternalInput").ap()
    out = nc.dram_tensor("out", [NB, F], f32, kind="ExternalOutput").ap()

    pid = nc.partition_id()

    with tile.TileContext(nc) as tc:
        with tc.tile_pool(name="dram", bufs=1, space="DRAM") as dram, \
             tc.tile_pool(name="persist", bufs=1) as pp:
            hg_in = dram.tile([NB, F], bf16)
            hg_out = dram.tile([N, F], bf16, addr_space="Shared")
            dg_in = dram.tile([128, KT], f32)
            dg_out = dram.tile([128 * N_CORES, KT], f32, addr_space="Shared")

            cs = pp.tile([128, KT], f32)   # per-core partial column sums

            # ---- phase 1: H_blk = X_blk @ W + b  (f32r matmuls) ----
            with tc.tile_pool(name="ph1", bufs=1) as p1, \
                 tc.tile_pool(name="ph1ps", bufs=1, space="PSUM") as ps1:
                xt_sb = p1.tile([128, FI_T * NB], f32)
                w_sb = p1.tile([128, FI_T * F], f32)
                for ki in range(FI_T):
                    nc.sync.dma_start(xt_sb[:, ki * NB:(ki + 1) * NB],
                                      xt[ki * 128:(ki + 1) * 128, :])
                    nc.sync.dma_start(w_sb[:, ki * F:(ki + 1) * F],
                                      w[ki * 128:(ki + 1) * 128, :])
                b_sb = p1.tile([128, F], f32)
                nc.sync.dma_start(b_sb[:], bfull)
                # round fp32 -> fp32r for the PE
                xt_r = p1.tile([128, FI_T * NB], f32r)
                nc.vector.tensor_copy(xt_r[:], xt_sb[:])
                w_r = p1.tile([128, FI_T * F], f32r)
                nc.vector.tensor_copy(w_r[:], w_sb[:])

                for nt in range(MT):
                    hp = ps1.tile([128, F], f32, tag="hp", bufs=2)
                    for ki in range(FI_T):
                        nc.tensor.matmul(
                            hp[:],
                            xt_r[:, ki * NB + nt * 128: ki * NB + (nt + 1) * 128],
                            w_r[:, ki * F:(ki + 1) * F],
                            start=(ki == 0), stop=(ki == FI_T - 1))
                    hb = p1.tile([128, F], bf16, tag="hb", bufs=2)
                    nc.vector.tensor_tensor(hb[:], hp[:], b_sb[:],
                                            mybir.AluOpType.add)
                    nc.sync.dma_start(hg_in[nt * 128:(nt + 1) * 128, :], hb[:])

            # ---- all-gather projected hidden ----
            nc.gpsimd.collective_compute(
                "AllGather", mybir.AluOpType.bypass,
                replica_groups=[list(range(N_CORES))],
                ins=[hg_in.opt()], outs=[hg_out.opt()],
            )

            # ---- phase 2: out_psum[m] += A_bin_kt[:, m].T @ H_kt ----
            with tc.tile_pool(name="ph2", bufs=1) as p2, \
                 tc.tile_pool(name="ph2ps", bufs=1, space="PSUM") as ps2:
                ones = p2.tile([128, NB], f32)
                nc.vector.memset(ones[:], 1.0)
                pms = []
                for m in range(MT):
                    pm = ps2.tile([128, F], f32, tag=f"pm{m}", name=f"pm{m}",
                                  bufs=1)
                    pms.append(pm)

                # PE warm-up: slow f32 (4-pass) junk matmuls keep the HAM
                # clock gate at 2.4 GHz through the barrier+AllGather stall.
                # Results land in the pm banks but the first real matmul's
                # start=True clears them.
                for j in range(N_DUMMY):
                    nc.tensor.matmul(pms[j % MT][:], ones[:, 0:128],
                                     ones[:, 0:F], start=True, stop=True)

                # A-tile loads (sync queue), prefetched ahead of the loop
                a_raws = []
                for kt in range(KT):
                    a_raw = p2.tile([128, NB], f32, tag="araw", bufs=8,
                                    name=f"araw{kt}")
                    nc.sync.dma_start(a_raw[:],
                                      at[kt * 128:(kt + 1) * 128, :])
                    a_raws.append(a_raw)

                for kt in range(KT):
                    # one DVE op: a_bin = (a_raw != 0) * 1.0 (bf16, exact),
                    # accum_out = free-dim sums = partial column sums of A
                    a_bin = p2.tile([128, NB], bf16, tag="abin", bufs=20,
                                    name=f"abin{kt}")
                    nc.vector.scalar_tensor_tensor(
                        a_bin[:], a_raws[kt][:], 0.0, ones[:],
                        mybir.AluOpType.not_equal, mybir.AluOpType.mult,
                        accum_out=cs[:, kt:kt + 1])
                    h_t = p2.tile([128, F], bf16, tag="ht", bufs=8,
                                  name=f"ht{kt}")
                    nc.scalar.dma_start(h_t[:],
                                        hg_out[kt * 128:(kt + 1) * 128, :])
                    for m in range(MT):
                        nc.tensor.matmul(
                            pms[m][:],
                            a_bin[:, m * 128:(m + 1) * 128],
                            h_t[:],
                            start=(kt == 0), stop=(kt == KT - 1))

                # ---- phase 3: degree + normalize ----
                nc.gpsimd.dma_start(dg_in[:], cs[:])
                nc.gpsimd.collective_compute(
                    "AllGather", mybir.AluOpType.bypass,
                    replica_groups=[list(range(N_CORES))],
                    ins=[dg_in.opt()], outs=[dg_out.opt()],
                )
                # pull each rank's partial for OUR column block (kt = pid*8+m)
                deg = p2.tile([128, MT], f32)
                prt0 = p2.tile([128, MT], f32, tag="prt", bufs=4, name="prt0")
                nc.gpsimd.dma_start(
                    prt0[:], dg_out[0:128, bass.ts(pid, MT)])
                nc.vector.tensor_copy(deg[:], prt0[:])
                for r in range(1, N_CORES):
                    prt = p2.tile([128, MT], f32, tag="prt", bufs=4,
                                  name=f"prt{r}")
                    nc.gpsimd.dma_start(
                        prt[:],
                        dg_out[r * 128:(r + 1) * 128, bass.ts(pid, MT)])
                    nc.vector.tensor_tensor(deg[:], deg[:], prt[:],
                                            mybir.AluOpType.add)
                rdeg = p2.tile([128, MT], f32)
                nc.vector.reciprocal(rdeg[:], deg[:])

                for m in range(MT):
                    o_sb = p2.tile([128, F], f32, tag="osb", bufs=2,
                                   name=f"osb{m}")
                    nc.scalar.mul(o_sb[:], pms[m][:], rdeg[:, m:m + 1])
                    nc.sync.dma_start(out[m * 128:(m + 1) * 128, :], o_sb[:])

    nc.compile()
    return nc


def _get_nc():
    if "nc" not in _cached:
        _cached["nc"] = _build()
    return _cached["nc"]


def kernel(input_features, adj, W, b):
    from concourse.bass_utils import run_bass_kernel_spmd

    x = np.ascontiguousarray(np.asarray(input_features, dtype=np.float32))
    a = np.asarray(adj, dtype=np.float32)
    wm = np.ascontiguousarray(np.asarray(W, dtype=np.float32))
    bv = np.asarray(b, dtype=np.float32)
    bfull = np.ascontiguousarray(np.broadcast_to(bv, (128, F)))

    nc = _get_nc()
    in_maps = []
    for k in range(N_CORES):
        blk = slice(k * NB, (k + 1) * NB)
        in_maps.append({
            "at": np.ascontiguousarray(a[blk, :].T),
            "xt": np.ascontiguousarray(x[blk, :].T),
            "w": wm,
            "bfull": bfull,
        })
    res = run_bass_kernel_spmd(nc, in_maps, core_ids=list(range(N_CORES)))
    return np.concatenate([res.results[k]["out"] for k in range(N_CORES)],
                          axis=0)
